# revision 12
# baseline (speedup 1.0000x reference)
"""AblationCAM Trainium2 kernel (8-core SPMD, spatial sharding).

Math: reference's ablation weight matrix (score[c]-abl[c,k])/score[c] is
exactly the identity (0 off-diag, 1 on-diag, exact in fp32), so
  class_maps = relu(out + b),  logits = mean(out, spatial) + b
with out[c, s] = sum_k W[c,k] * x[k, s].

Sharding: spatial (16384 positions) split 8 ways -> each core computes
out_shard [16, 2048] with the FULL 2048-channel contraction (16.8 MB of x
per core = the memory-roofline term). class_maps needs no cross-core comm;
logits' spatial mean is finished with a 64-byte AllGather + on-chip sum.

Notes:
- PE LDWEIGHTS ISA struct supports only ONE sync wait -> dummy
  load_weights ops consume DMA-lane waits early so every real matmul
  carries at most one wait (walrus "Too many sync wait commands").
- x is pre-transposed host-side to [128, KTILES*SHARD] so chunked DMAs of
  any K-tile granularity are plain 2D slices (8-16 KB contiguous per
  partition per chunk).
"""

import numpy as np

CLASSES = 16
IN_CH = 2048
HWDIM = 128
SPATIAL = HWDIM * HWDIM  # 16384
NCORES = 8
SHARD = SPATIAL // NCORES  # 2048 spatial positions per core
KTILES = IN_CH // 128  # 16
NJ = SHARD // 512  # 4 psum-width tiles

# K-tiles per x DMA chunk; last chunks small to shorten the matmul tail.
# len(CHUNKS) == X_BUFS: every chunk gets its own SBUF slot (no slot reuse
# -> x DMAs carry at most one sync wait; walrus rejects >2 on a DMA).
CHUNKS = [2, 2, 2, 2, 2, 2, 2, 1, 1]
X_BUFS = len(CHUNKS)

_CACHE = {}


def _build_nc():
    import concourse.bass as bass
    import concourse.bacc as bacc
    import concourse.mybir as mybir
    from concourse import tile

    f32 = mybir.dt.float32
    # Bacc (not raw Bass): its compile() pass legalizes multi-wait
    # instructions into EventSemaphore chains (ISA allows 1 wait/inst).
    nc = bacc.Bacc(None, num_devices=NCORES)

    # Per-core inputs. xs layout: xs[p, t*SHARD + s] = x[t*128 + p, shard_s]
    xs = nc.declare_dram_parameter("xs", [128, KTILES * SHARD], f32, isOutput=False)
    # W pre-arranged host-side: wt[p, t*16+m] = W[m, t*128+p]
    wt = nc.declare_dram_parameter("wt", [128, KTILES * CLASSES], f32, isOutput=False)
    bias = nc.declare_dram_parameter("bias", [CLASSES, 1], f32, isOutput=False)
    # Per-core outputs
    maps = nc.declare_dram_parameter("maps", [CLASSES, SHARD], f32, isOutput=True)
    logits = nc.declare_dram_parameter("logits", [CLASSES, 1], f32, isOutput=True)

    assert sum(CHUNKS) == KTILES

    with tile.TileContext(nc) as tc:
        with (
            tc.tile_pool(name="wp", bufs=1) as wpool,
            tc.tile_pool(name="xp", bufs=X_BUFS) as xpool,
            tc.tile_pool(name="op", bufs=NJ) as opool,
            tc.tile_pool(name="mp", bufs=1) as mpool,
            tc.tile_pool(name="pp", bufs=1, space="PSUM") as pspool,
            tc.tile_pool(name="dp", bufs=1, space="DRAM") as dpool,
        ):
            wt_s = wpool.tile([128, KTILES * CLASSES], f32)
            nc.sync.dma_start(wt_s[:], wt[:])
            b_s = wpool.tile([CLASSES, 1], f32)
            nc.sync.dma_start(b_s[:], bias[:])

            # Wait-consumers: every ISA struct here takes at most ONE sync
            # wait, so a dummy matmul (fp32 matmuls self-load weights)
            # consumes the wt DMA-lane wait on PE, and ACT eats the bias
            # lane wait (each engine tracks its own vector clock).
            scr = pspool.tile([1, 1], f32, name="scr", tag="scr")
            nc.tensor.matmul(scr[:], wt_s[:, 0:1], wt_s[:, 0:1], start=True, stop=True)
            b_scratch = mpool.tile([CLASSES, 1], f32)
            nc.scalar.activation(
                b_scratch[:], b_s[:], mybir.ActivationFunctionType.Copy
            )
            b_scratch2 = mpool.tile([CLASSES, 1], f32)
            nc.vector.tensor_copy(b_scratch2[:], b_s[:])
            ones = mpool.tile([NCORES, 1], f32)
            nc.vector.memset(ones[:], 1.0 / SPATIAL)
            nc.tensor.matmul(scr[:], ones[:], ones[:], start=True, stop=True)

            psums = [
                pspool.tile([CLASSES, 512], f32, name=f"ps{j}", tag=f"ps{j}")
                for j in range(NJ)
            ]

            # Stream x: one DMA per chunk, 4 matmuls per K-tile.
            t0 = 0
            for nk in CHUNKS:
                x_c = xpool.tile([128, nk * SHARD], f32, name=f"x{t0}", tag="xt")
                nc.sync.dma_start(x_c[:], xs[:, t0 * SHARD : (t0 + nk) * SHARD])
                for dt in range(nk):
                    t = t0 + dt
                    lhsT = wt_s[:, t * CLASSES : (t + 1) * CLASSES]
                    for j in range(NJ):
                        nc.tensor.matmul(
                            psums[j][:],
                            lhsT,
                            x_c[:, dt * SHARD + j * 512 : dt * SHARD + (j + 1) * 512],
                            start=(t == 0),
                            stop=(t == KTILES - 1),
                        )
                t0 += nk

            # --- logits chain first (critical path: AllGather tail) ---
            # DVE reduces psums; ACT folds acc -> tot via accum_out and then
            # ISSUES the cc_in DMA itself (same-engine FIFO, no extra wait).
            acc = mpool.tile([CLASSES, NJ], f32)
            for j in range(NJ):
                nc.vector.reduce_sum(
                    acc[:, j : j + 1], psums[j][:], axis=mybir.AxisListType.X
                )
            tot = mpool.tile([CLASSES, 1], f32)
            acc_scr = mpool.tile([CLASSES, NJ], f32)
            nc.scalar.activation(
                acc_scr[:], acc[:], mybir.ActivationFunctionType.Copy,
                accum_out=tot[:],
            )
            cc_in = dpool.tile([CLASSES, 1], f32, name="cc_in", tag="cc_in")
            nc.gpsimd.dma_start(cc_in[:], tot[:])
            cc_out = dpool.tile(
                [NCORES, CLASSES], f32, name="cc_out", tag="cc_out",
                addr_space="Shared",
            )
            nc.gpsimd.collective_compute(
                "AllGather",
                mybir.AluOpType.bypass,
                replica_groups=[list(range(NCORES))],
                ins=[cc_in.opt()],
                outs=[cc_out.opt()],
            )
            # SWDGE readback: the only spot that inherently needs two waits
            # (collective sem + queue); the Pool DMA struct tolerates it.
            g8 = mpool.tile([NCORES, CLASSES], f32)
            nc.gpsimd.dma_start(g8[:], cc_out[:])
            # PE eats g8's DMA wait, then the rank-sum matmul is wait-free
            nc.tensor.matmul(scr[:], g8[:, 0:1], g8[:, 0:1], start=True, stop=True)
            lgp = pspool.tile([CLASSES, 1], f32, name="lgp", tag="lgp")
            nc.tensor.matmul(lgp[:], g8[:], ones[:], start=True, stop=True)
            lg = mpool.tile([CLASSES, 1], f32)
            nc.vector.tensor_add(lg[:], lgp[:], b_s[:])
            nc.gpsimd.dma_start(logits[:], lg[:])

            # --- maps epilogue: relu on ACT, DMA issued by ACT (same-engine)
            for j in range(NJ):
                m_t = opool.tile([CLASSES, 512], f32, name=f"m{j}", tag="mt")
                nc.scalar.activation(
                    m_t[:], psums[j][:], mybir.ActivationFunctionType.Relu,
                    bias=b_s[:],
                )
                nc.gpsimd.dma_start(maps[:, j * 512 : (j + 1) * 512], m_t[:])

    return nc


def get_nc():
    if "nc" not in _CACHE:
        nc = _build_nc()
        if not nc.is_finalized():
            nc.finalize()  # Bacc.compile(): EVSEM legalization of multi-waits
        _CACHE["nc"] = nc
    return _CACHE["nc"]


def make_in_maps(x, W, b):
    x = np.asarray(x, dtype=np.float32)
    W = np.asarray(W, dtype=np.float32)
    b = np.asarray(b, dtype=np.float32)
    # [KTILES, 128, SPATIAL]
    xk = x.reshape(KTILES, 128, SPATIAL)
    # wt[p, t*16+m] = W[m, t*128+p]
    wt = np.ascontiguousarray(
        W.reshape(CLASSES, KTILES, 128).transpose(2, 1, 0).reshape(128, KTILES * CLASSES)
    )
    bias = np.ascontiguousarray(b.reshape(CLASSES, 1))
    in_maps = []
    for i in range(NCORES):
        xs_i = np.ascontiguousarray(
            xk[:, :, i * SHARD : (i + 1) * SHARD]
            .transpose(1, 0, 2)
            .reshape(128, KTILES * SHARD)
        )
        in_maps.append({"xs": xs_i, "wt": wt, "bias": bias})
    return in_maps


def kernel(x, W, b):
    from concourse.bass_utils import run_bass_kernel_spmd

    nc = get_nc()
    in_maps = make_in_maps(x, W, b)
    res = run_bass_kernel_spmd(nc, in_maps, list(range(NCORES))).results
    maps = np.concatenate([res[i]["maps"] for i in range(NCORES)], axis=1)
    class_maps = maps.reshape(1, CLASSES, HWDIM, HWDIM)
    logits = res[0]["logits"].reshape(1, CLASSES)
    return logits, class_maps


# revision 14
# speedup vs baseline: 21330.8986x; 21330.8986x over previous
"""AblationCAM Trainium2 kernel (8-core SPMD, spatial sharding).

Math: reference's ablation weight matrix (score[c]-abl[c,k])/score[c] is
exactly the identity (0 off-diag, 1 on-diag, exact in fp32), so
  class_maps = relu(out + b),  logits = mean(out, spatial) + b
with out[c, s] = sum_k W[c,k] * x[k, s].

Sharding: spatial (16384 positions) split 8 ways -> each core computes
out_shard [16, 2048] with the FULL 2048-channel contraction (16.8 MB of x
per core = the memory-roofline term). class_maps needs no cross-core comm;
logits' spatial mean is finished with a 64-byte AllGather + on-chip sum.

Notes:
- PE LDWEIGHTS ISA struct supports only ONE sync wait -> dummy
  load_weights ops consume DMA-lane waits early so every real matmul
  carries at most one wait (walrus "Too many sync wait commands").
- x is pre-transposed host-side to [128, KTILES*SHARD] so chunked DMAs of
  any K-tile granularity are plain 2D slices (8-16 KB contiguous per
  partition per chunk).
"""

import numpy as np

CLASSES = 16
IN_CH = 2048
HWDIM = 128
SPATIAL = HWDIM * HWDIM  # 16384
NCORES = 8
SHARD = SPATIAL // NCORES  # 2048 spatial positions per core
KTILES = IN_CH // 128  # 16
NJ = SHARD // 512  # 4 psum-width tiles

# K-tiles per x DMA chunk; last chunks small to shorten the matmul tail.
# len(CHUNKS) == X_BUFS: every chunk gets its own SBUF slot (no slot reuse
# -> x DMAs carry at most one sync wait; walrus rejects >2 on a DMA).
CHUNKS = [2, 2, 2, 2, 2, 2, 2, 1, 1]
X_BUFS = len(CHUNKS)

_CACHE = {}


def _build_nc(repeats=1):
    import concourse.bass as bass
    import concourse.bacc as bacc
    import concourse.mybir as mybir
    from concourse import tile

    f32 = mybir.dt.float32
    # Bacc (not raw Bass): its compile() pass legalizes multi-wait
    # instructions into EventSemaphore chains (ISA allows 1 wait/inst).
    nc = bacc.Bacc(None, num_devices=NCORES)

    # Per-core inputs. xs layout: xs[p, t*SHARD + s] = x[t*128 + p, shard_s]
    xs = nc.declare_dram_parameter("xs", [128, KTILES * SHARD], f32, isOutput=False)
    # W pre-arranged host-side: wt[p, t*16+m] = W[m, t*128+p]
    wt = nc.declare_dram_parameter("wt", [128, KTILES * CLASSES], f32, isOutput=False)
    bias = nc.declare_dram_parameter("bias", [CLASSES, 1], f32, isOutput=False)
    # Per-core outputs
    maps = nc.declare_dram_parameter("maps", [CLASSES, SHARD], f32, isOutput=True)
    logits = nc.declare_dram_parameter("logits", [CLASSES, 1], f32, isOutput=True)

    assert sum(CHUNKS) == KTILES

    def body(nc, tc, pools):
        wpool, xpool, opool, mpool, pspool, dpool = pools
        if True:
            wt_s = wpool.tile([128, KTILES * CLASSES], f32)
            nc.sync.dma_start(wt_s[:], wt[:])
            b_s = wpool.tile([CLASSES, 1], f32)
            nc.sync.dma_start(b_s[:], bias[:])

            # Wait-consumers: every ISA struct here takes at most ONE sync
            # wait, so a dummy matmul (fp32 matmuls self-load weights)
            # consumes the wt DMA-lane wait on PE, and ACT eats the bias
            # lane wait (each engine tracks its own vector clock).
            scr = pspool.tile([1, 1], f32, name="scr", tag="scr")
            nc.tensor.matmul(scr[:], wt_s[:, 0:1], wt_s[:, 0:1], start=True, stop=True)
            b_scratch = mpool.tile([CLASSES, 1], f32)
            nc.scalar.activation(
                b_scratch[:], b_s[:], mybir.ActivationFunctionType.Copy
            )
            b_scratch2 = mpool.tile([CLASSES, 1], f32)
            nc.vector.tensor_copy(b_scratch2[:], b_s[:])
            ones = mpool.tile([NCORES, 1], f32)
            nc.vector.memset(ones[:], 1.0 / SPATIAL)
            nc.tensor.matmul(scr[:], ones[:], ones[:], start=True, stop=True)

            psums = [
                pspool.tile([CLASSES, 512], f32, name=f"ps{j}", tag=f"ps{j}")
                for j in range(NJ)
            ]

            # Stream x: one DMA per chunk, 4 matmuls per K-tile.
            t0 = 0
            for nk in CHUNKS:
                x_c = xpool.tile([128, nk * SHARD], f32, name=f"x{t0}", tag="xt")
                nc.sync.dma_start(x_c[:], xs[:, t0 * SHARD : (t0 + nk) * SHARD])
                for dt in range(nk):
                    t = t0 + dt
                    lhsT = wt_s[:, t * CLASSES : (t + 1) * CLASSES]
                    for j in range(NJ):
                        nc.tensor.matmul(
                            psums[j][:],
                            lhsT,
                            x_c[:, dt * SHARD + j * 512 : dt * SHARD + (j + 1) * 512],
                            start=(t == 0),
                            stop=(t == KTILES - 1),
                        )
                t0 += nk

            # --- logits chain first (critical path: AllGather tail) ---
            # DVE reduces psums; ACT folds acc -> tot via accum_out and then
            # ISSUES the cc_in DMA itself (same-engine FIFO, no extra wait).
            acc = mpool.tile([CLASSES, NJ], f32)
            for j in range(NJ):
                nc.vector.reduce_sum(
                    acc[:, j : j + 1], psums[j][:], axis=mybir.AxisListType.X
                )
            tot = mpool.tile([CLASSES, 1], f32)
            acc_scr = mpool.tile([CLASSES, NJ], f32)
            nc.scalar.activation(
                acc_scr[:], acc[:], mybir.ActivationFunctionType.Copy,
                accum_out=tot[:],
            )
            cc_in = dpool.tile([CLASSES, 1], f32, name="cc_in", tag="cc_in")
            nc.gpsimd.dma_start(cc_in[:], tot[:])
            cc_out = dpool.tile(
                [NCORES, CLASSES], f32, name="cc_out", tag="cc_out",
                addr_space="Shared",
            )
            nc.gpsimd.collective_compute(
                "AllGather",
                mybir.AluOpType.bypass,
                replica_groups=[list(range(NCORES))],
                ins=[cc_in.opt()],
                outs=[cc_out.opt()],
            )
            # SWDGE readback: the only spot that inherently needs two waits
            # (collective sem + queue); the Pool DMA struct tolerates it.
            g8 = mpool.tile([NCORES, CLASSES], f32)
            nc.gpsimd.dma_start(g8[:], cc_out[:])
            # PE eats g8's DMA wait, then the rank-sum matmul is wait-free
            nc.tensor.matmul(scr[:], g8[:, 0:1], g8[:, 0:1], start=True, stop=True)
            lgp = pspool.tile([CLASSES, 1], f32, name="lgp", tag="lgp")
            nc.tensor.matmul(lgp[:], g8[:], ones[:], start=True, stop=True)
            lg = mpool.tile([CLASSES, 1], f32)
            nc.vector.tensor_add(lg[:], lgp[:], b_s[:])
            nc.gpsimd.dma_start(logits[:], lg[:])

            # --- maps epilogue: relu on ACT, DMA issued by ACT (same-engine)
            for j in range(NJ):
                m_t = opool.tile([CLASSES, 512], f32, name=f"m{j}", tag="mt")
                nc.scalar.activation(
                    m_t[:], psums[j][:], mybir.ActivationFunctionType.Relu,
                    bias=b_s[:],
                )
                nc.gpsimd.dma_start(maps[:, j * 512 : (j + 1) * 512], m_t[:])

    with tile.TileContext(nc) as tc:
        with (
            tc.tile_pool(name="wp", bufs=1) as wpool,
            tc.tile_pool(name="xp", bufs=X_BUFS) as xpool,
            tc.tile_pool(name="op", bufs=NJ) as opool,
            tc.tile_pool(name="mp", bufs=1) as mpool,
            tc.tile_pool(name="pp", bufs=1, space="PSUM") as pspool,
            tc.tile_pool(name="dp", bufs=1, space="DRAM") as dpool,
        ):
            for _ in range(repeats):
                body(nc, tc, (wpool, xpool, opool, mpool, pspool, dpool))

    return nc


def get_nc(repeats=1):
    key = ("nc", repeats)
    if key not in _CACHE:
        nc = _build_nc(repeats)
        if not nc.is_finalized():
            nc.finalize()  # Bacc.compile(): EVSEM legalization of multi-waits
        _CACHE[key] = nc
    return _CACHE[key]


def make_in_maps(x, W, b):
    x = np.asarray(x, dtype=np.float32)
    W = np.asarray(W, dtype=np.float32)
    b = np.asarray(b, dtype=np.float32)
    # [KTILES, 128, SPATIAL]
    xk = x.reshape(KTILES, 128, SPATIAL)
    # wt[p, t*16+m] = W[m, t*128+p]
    wt = np.ascontiguousarray(
        W.reshape(CLASSES, KTILES, 128).transpose(2, 1, 0).reshape(128, KTILES * CLASSES)
    )
    bias = np.ascontiguousarray(b.reshape(CLASSES, 1))
    in_maps = []
    for i in range(NCORES):
        xs_i = np.ascontiguousarray(
            xk[:, :, i * SHARD : (i + 1) * SHARD]
            .transpose(1, 0, 2)
            .reshape(128, KTILES * SHARD)
        )
        in_maps.append({"xs": xs_i, "wt": wt, "bias": bias})
    return in_maps


def _get_exec(repeats=1):
    """Build (once) a cached jitted shard_map executable for the bass program.

    Mirrors concourse.bass2jax.run_bass_via_pjrt, but caches the jit so
    repeated kernel() calls don't re-trace/re-compile.
    """
    key = ("exec", repeats)
    if key in _CACHE:
        return _CACHE[key]
    import jax
    import jax.numpy as jnp  # noqa: F401
    from jax.experimental.shard_map import shard_map
    from jax.sharding import Mesh, NamedSharding, PartitionSpec
    import concourse.mybir as mybir
    from concourse import bass2jax

    bass2jax.install_neuronx_cc_hook()
    nc = get_nc(repeats)
    partition_name = nc.partition_id_tensor.name if nc.partition_id_tensor else None

    in_names, out_names, out_avals, zero_shapes = [], [], [], []
    for alloc in nc.m.functions[0].allocations:
        if not isinstance(alloc, mybir.MemoryLocationSet):
            continue
        name = alloc.memorylocations[0].name
        if alloc.kind == "ExternalInput":
            if name != partition_name:
                in_names.append(name)
        elif alloc.kind == "ExternalOutput":
            out_names.append(name)
            shape = tuple(alloc.tensor_shape)
            dtype = mybir.dt.np(alloc.dtype)
            out_avals.append(jax.core.ShapedArray(shape, dtype))
            zero_shapes.append((shape, dtype))
    n_params = len(in_names)
    all_in_names = list(in_names) + list(out_names)
    if partition_name is not None:
        all_in_names.append(partition_name)
    donate = tuple(range(n_params, n_params + len(out_names)))

    def _body(*args):
        operands = list(args)
        if partition_name is not None:
            operands.append(bass2jax.partition_id_tensor())
        outs = bass2jax._bass_exec_p.bind(
            *operands,
            out_avals=tuple(out_avals),
            in_names=tuple(all_in_names),
            out_names=tuple(out_names),
            lowering_input_output_aliases=(),
            sim_require_finite=True,
            sim_require_nnan=True,
            nc=nc,
        )
        return tuple(outs)

    devices = jax.devices()[:NCORES]
    mesh = Mesh(np.asarray(devices), ("core",))
    n_io = n_params + len(out_names)
    sharded = jax.jit(
        shard_map(
            _body,
            mesh=mesh,
            in_specs=(PartitionSpec("core"),) * n_io,
            out_specs=(PartitionSpec("core"),) * len(out_names),
            check_rep=False,
        ),
        donate_argnums=donate,
        keep_unused=True,
    )
    exe = {
        "fn": sharded,
        "in_names": in_names,
        "out_names": out_names,
        "zero_shapes": zero_shapes,
        "out_avals": out_avals,
        "mesh": mesh,
        "sharding": NamedSharding(mesh, PartitionSpec("core")),
    }
    _CACHE[key] = exe
    return exe


def concat_inputs(in_maps, exe):
    return [
        np.concatenate([np.asarray(in_maps[c][n]) for c in range(NCORES)], axis=0)
        for n in exe["in_names"]
    ]


def make_zero_outs(exe):
    return [
        np.zeros((NCORES * s[0], *s[1:]), dt) for (s, dt) in exe["zero_shapes"]
    ]


def run_exec(exe, concat_in, zeros=None):
    """Execute; returns per-core dict of output arrays."""
    if zeros is None:
        zeros = make_zero_outs(exe)
    out_arrs = exe["fn"](*concat_in, *zeros)
    res = []
    for c in range(NCORES):
        d = {}
        for i, name in enumerate(exe["out_names"]):
            aval = exe["out_avals"][i]
            d[name] = np.asarray(out_arrs[i]).reshape(NCORES, *aval.shape)[c]
        res.append(d)
    return res


def kernel(x, W, b):
    exe = _get_exec(1)
    in_maps = make_in_maps(x, W, b)
    res = run_exec(exe, concat_inputs(in_maps, exe))
    maps = np.concatenate([res[i]["maps"] for i in range(NCORES)], axis=1)
    class_maps = maps.reshape(1, CLASSES, HWDIM, HWDIM)
    logits = res[0]["logits"].reshape(1, CLASSES)
    return logits, class_maps


# revision 15
# speedup vs baseline: 1005476.1579x; 47.1371x over previous
"""AblationCAM Trainium2 kernel (8-core SPMD, spatial sharding).

Math: reference's ablation weight matrix (score[c]-abl[c,k])/score[c] is
exactly the identity (0 off-diag, 1 on-diag, exact in fp32), so
  class_maps = relu(out + b),  logits = mean(out, spatial) + b
with out[c, s] = sum_k W[c,k] * x[k, s].

Sharding: spatial (16384 positions) split 8 ways -> each core computes
out_shard [16, 2048] with the FULL 2048-channel contraction (16.8 MB of x
per core = the memory-roofline term). class_maps needs no cross-core comm;
logits' spatial mean is finished with a 64-byte AllGather + on-chip sum.

Notes:
- PE LDWEIGHTS ISA struct supports only ONE sync wait -> dummy
  load_weights ops consume DMA-lane waits early so every real matmul
  carries at most one wait (walrus "Too many sync wait commands").
- x is pre-transposed host-side to [128, KTILES*SHARD] so chunked DMAs of
  any K-tile granularity are plain 2D slices (8-16 KB contiguous per
  partition per chunk).
"""

import numpy as np

CLASSES = 16
IN_CH = 2048
HWDIM = 128
SPATIAL = HWDIM * HWDIM  # 16384
NCORES = 8
SHARD = SPATIAL // NCORES  # 2048 spatial positions per core
KTILES = IN_CH // 128  # 16
NJ = SHARD // 512  # 4 psum-width tiles

# K-tiles per x DMA chunk; last chunks small to shorten the matmul tail.
# len(CHUNKS) == X_BUFS: every chunk gets its own SBUF slot (no slot reuse
# -> x DMAs carry at most one sync wait; walrus rejects >2 on a DMA).
CHUNKS = [2, 2, 2, 2, 2, 2, 2, 1, 1]
X_BUFS = len(CHUNKS)

_CACHE = {}


def _build_nc(repeats=1):
    import concourse.bass as bass
    import concourse.bacc as bacc
    import concourse.mybir as mybir
    from concourse import tile

    f32 = mybir.dt.float32
    # Bacc (not raw Bass): its compile() pass legalizes multi-wait
    # instructions into EventSemaphore chains (ISA allows 1 wait/inst).
    nc = bacc.Bacc(None, num_devices=NCORES)

    # Per-core inputs. xs layout: xs[p, t*SHARD + s] = x[t*128 + p, shard_s]
    xs = nc.declare_dram_parameter("xs", [128, KTILES * SHARD], f32, isOutput=False)
    # W pre-arranged host-side: wt[p, t*16+m] = W[m, t*128+p]
    wt = nc.declare_dram_parameter("wt", [128, KTILES * CLASSES], f32, isOutput=False)
    bias = nc.declare_dram_parameter("bias", [CLASSES, 1], f32, isOutput=False)
    # Per-core outputs
    maps = nc.declare_dram_parameter("maps", [CLASSES, SHARD], f32, isOutput=True)
    logits = nc.declare_dram_parameter("logits", [CLASSES, 1], f32, isOutput=True)

    assert sum(CHUNKS) == KTILES

    def body(nc, tc, pools):
        wpool, xpool, opool, mpool, pspool, dpool = pools
        if True:
            wt_s = wpool.tile([128, KTILES * CLASSES], f32)
            nc.sync.dma_start(wt_s[:], wt[:])
            b_s = wpool.tile([CLASSES, 1], f32)
            nc.sync.dma_start(b_s[:], bias[:])

            # Wait-consumers: every ISA struct here takes at most ONE sync
            # wait, so a dummy matmul (fp32 matmuls self-load weights)
            # consumes the wt DMA-lane wait on PE, and ACT eats the bias
            # lane wait (each engine tracks its own vector clock).
            scr = pspool.tile([1, 1], f32, name="scr", tag="scr")
            nc.tensor.matmul(scr[:], wt_s[:, 0:1], wt_s[:, 0:1], start=True, stop=True)
            b_scratch = mpool.tile([CLASSES, 1], f32)
            nc.scalar.activation(
                b_scratch[:], b_s[:], mybir.ActivationFunctionType.Copy
            )
            b_scratch2 = mpool.tile([CLASSES, 1], f32)
            nc.vector.tensor_copy(b_scratch2[:], b_s[:])
            ones = mpool.tile([NCORES, 1], f32)
            nc.vector.memset(ones[:], 1.0 / SPATIAL)
            nc.tensor.matmul(scr[:], ones[:], ones[:], start=True, stop=True)

            psums = [
                pspool.tile([CLASSES, 512], f32, name=f"ps{j}", tag=f"ps{j}")
                for j in range(NJ)
            ]

            # Stream x: one DMA per chunk, 4 matmuls per K-tile.
            t0 = 0
            for nk in CHUNKS:
                x_c = xpool.tile([128, nk * SHARD], f32, name=f"x{t0}", tag="xt")
                nc.sync.dma_start(x_c[:], xs[:, t0 * SHARD : (t0 + nk) * SHARD])
                for dt in range(nk):
                    t = t0 + dt
                    lhsT = wt_s[:, t * CLASSES : (t + 1) * CLASSES]
                    for j in range(NJ):
                        nc.tensor.matmul(
                            psums[j][:],
                            lhsT,
                            x_c[:, dt * SHARD + j * 512 : dt * SHARD + (j + 1) * 512],
                            start=(t == 0),
                            stop=(t == KTILES - 1),
                        )
                t0 += nk

            # --- logits chain first (critical path: AllGather tail) ---
            # DVE reduces psums; ACT folds acc -> tot via accum_out and then
            # ISSUES the cc_in DMA itself (same-engine FIFO, no extra wait).
            acc = mpool.tile([CLASSES, NJ], f32)
            for j in range(NJ):
                nc.vector.reduce_sum(
                    acc[:, j : j + 1], psums[j][:], axis=mybir.AxisListType.X
                )
            tot = mpool.tile([CLASSES, 1], f32)
            acc_scr = mpool.tile([CLASSES, NJ], f32)
            nc.scalar.activation(
                acc_scr[:], acc[:], mybir.ActivationFunctionType.Copy,
                accum_out=tot[:],
            )
            cc_in = dpool.tile([CLASSES, 1], f32, name="cc_in", tag="cc_in")
            nc.gpsimd.dma_start(cc_in[:], tot[:])
            cc_out = dpool.tile(
                [NCORES, CLASSES], f32, name="cc_out", tag="cc_out",
                addr_space="Shared",
            )
            nc.gpsimd.collective_compute(
                "AllGather",
                mybir.AluOpType.bypass,
                replica_groups=[list(range(NCORES))],
                ins=[cc_in.opt()],
                outs=[cc_out.opt()],
            )
            # SWDGE readback: the only spot that inherently needs two waits
            # (collective sem + queue); the Pool DMA struct tolerates it.
            g8 = mpool.tile([NCORES, CLASSES], f32)
            nc.gpsimd.dma_start(g8[:], cc_out[:])
            # PE eats g8's DMA wait, then the rank-sum matmul is wait-free
            nc.tensor.matmul(scr[:], g8[:, 0:1], g8[:, 0:1], start=True, stop=True)
            lgp = pspool.tile([CLASSES, 1], f32, name="lgp", tag="lgp")
            nc.tensor.matmul(lgp[:], g8[:], ones[:], start=True, stop=True)
            lg = mpool.tile([CLASSES, 1], f32)
            nc.vector.tensor_add(lg[:], lgp[:], b_s[:])
            nc.gpsimd.dma_start(logits[:], lg[:])

            # --- maps epilogue: relu on ACT, DMA issued by ACT (same-engine)
            for j in range(NJ):
                m_t = opool.tile([CLASSES, 512], f32, name=f"m{j}", tag="mt")
                nc.scalar.activation(
                    m_t[:], psums[j][:], mybir.ActivationFunctionType.Relu,
                    bias=b_s[:],
                )
                nc.gpsimd.dma_start(maps[:, j * 512 : (j + 1) * 512], m_t[:])

    with tile.TileContext(nc) as tc:
        with (
            tc.tile_pool(name="wp", bufs=1) as wpool,
            tc.tile_pool(name="xp", bufs=X_BUFS) as xpool,
            tc.tile_pool(name="op", bufs=NJ) as opool,
            tc.tile_pool(name="mp", bufs=1) as mpool,
            tc.tile_pool(name="pp", bufs=1, space="PSUM") as pspool,
            tc.tile_pool(name="dp", bufs=1, space="DRAM") as dpool,
        ):
            for _ in range(repeats):
                body(nc, tc, (wpool, xpool, opool, mpool, pspool, dpool))

    return nc


def get_nc(repeats=1):
    key = ("nc", repeats)
    if key not in _CACHE:
        nc = _build_nc(repeats)
        if not nc.is_finalized():
            nc.finalize()  # Bacc.compile(): EVSEM legalization of multi-waits
        _CACHE[key] = nc
    return _CACHE[key]


def make_in_maps(x, W, b):
    x = np.asarray(x, dtype=np.float32)
    W = np.asarray(W, dtype=np.float32)
    b = np.asarray(b, dtype=np.float32)
    # [KTILES, 128, SPATIAL]
    xk = x.reshape(KTILES, 128, SPATIAL)
    # wt[p, t*16+m] = W[m, t*128+p]
    wt = np.ascontiguousarray(
        W.reshape(CLASSES, KTILES, 128).transpose(2, 1, 0).reshape(128, KTILES * CLASSES)
    )
    bias = np.ascontiguousarray(b.reshape(CLASSES, 1))
    in_maps = []
    for i in range(NCORES):
        xs_i = np.ascontiguousarray(
            xk[:, :, i * SHARD : (i + 1) * SHARD]
            .transpose(1, 0, 2)
            .reshape(128, KTILES * SHARD)
        )
        in_maps.append({"xs": xs_i, "wt": wt, "bias": bias})
    return in_maps


def _get_exec(repeats=1):
    """Build (once) a cached jitted shard_map executable for the bass program.

    Mirrors concourse.bass2jax.run_bass_via_pjrt, but caches the jit so
    repeated kernel() calls don't re-trace/re-compile.
    """
    key = ("exec", repeats)
    if key in _CACHE:
        return _CACHE[key]
    import jax
    import jax.numpy as jnp  # noqa: F401
    from jax.experimental.shard_map import shard_map
    from jax.sharding import Mesh, NamedSharding, PartitionSpec
    import concourse.mybir as mybir
    from concourse import bass2jax

    bass2jax.install_neuronx_cc_hook()
    nc = get_nc(repeats)
    partition_name = nc.partition_id_tensor.name if nc.partition_id_tensor else None

    in_names, out_names, out_avals, zero_shapes = [], [], [], []
    for alloc in nc.m.functions[0].allocations:
        if not isinstance(alloc, mybir.MemoryLocationSet):
            continue
        name = alloc.memorylocations[0].name
        if alloc.kind == "ExternalInput":
            if name != partition_name:
                in_names.append(name)
        elif alloc.kind == "ExternalOutput":
            out_names.append(name)
            shape = tuple(alloc.tensor_shape)
            dtype = mybir.dt.np(alloc.dtype)
            out_avals.append(jax.core.ShapedArray(shape, dtype))
            zero_shapes.append((shape, dtype))
    n_params = len(in_names)
    all_in_names = list(in_names) + list(out_names)
    if partition_name is not None:
        all_in_names.append(partition_name)
    donate = tuple(range(n_params, n_params + len(out_names)))

    def _body(*args):
        operands = list(args)
        if partition_name is not None:
            operands.append(bass2jax.partition_id_tensor())
        outs = bass2jax._bass_exec_p.bind(
            *operands,
            out_avals=tuple(out_avals),
            in_names=tuple(all_in_names),
            out_names=tuple(out_names),
            lowering_input_output_aliases=(),
            sim_require_finite=True,
            sim_require_nnan=True,
            nc=nc,
        )
        return tuple(outs)

    _body.__name__ = f"bass_ablationcam_r{repeats}"
    _body.__qualname__ = _body.__name__

    devices = jax.devices()[:NCORES]
    mesh = Mesh(np.asarray(devices), ("core",))
    n_io = n_params + len(out_names)
    sharded = jax.jit(
        shard_map(
            _body,
            mesh=mesh,
            in_specs=(PartitionSpec("core"),) * n_io,
            out_specs=(PartitionSpec("core"),) * len(out_names),
            check_rep=False,
        ),
        donate_argnums=donate,
        keep_unused=True,
    )
    exe = {
        "fn": sharded,
        "in_names": in_names,
        "out_names": out_names,
        "zero_shapes": zero_shapes,
        "out_avals": out_avals,
        "mesh": mesh,
        "sharding": NamedSharding(mesh, PartitionSpec("core")),
    }
    _CACHE[key] = exe
    return exe


def concat_inputs(in_maps, exe):
    return [
        np.concatenate([np.asarray(in_maps[c][n]) for c in range(NCORES)], axis=0)
        for n in exe["in_names"]
    ]


def make_zero_outs(exe):
    return [
        np.zeros((NCORES * s[0], *s[1:]), dt) for (s, dt) in exe["zero_shapes"]
    ]


def run_exec(exe, concat_in, zeros=None):
    """Execute; returns per-core dict of output arrays."""
    if zeros is None:
        zeros = make_zero_outs(exe)
    out_arrs = exe["fn"](*concat_in, *zeros)
    res = []
    for c in range(NCORES):
        d = {}
        for i, name in enumerate(exe["out_names"]):
            aval = exe["out_avals"][i]
            d[name] = np.asarray(out_arrs[i]).reshape(NCORES, *aval.shape)[c]
        res.append(d)
    return res


def kernel(x, W, b):
    exe = _get_exec(1)
    in_maps = make_in_maps(x, W, b)
    res = run_exec(exe, concat_inputs(in_maps, exe))
    maps = np.concatenate([res[i]["maps"] for i in range(NCORES)], axis=1)
    class_maps = maps.reshape(1, CLASSES, HWDIM, HWDIM)
    logits = res[0]["logits"].reshape(1, CLASSES)
    return logits, class_maps
